# revision 29
# baseline (speedup 1.0000x reference)
"""Trainium2 Bass kernel: ResNet BasicBlock (conv3x3-BN-ReLU-mask-conv3x3-mask-BN-residual-ReLU).

Problem shape: x[4096, 64, 7, 7], both convs 64->64 3x3 pad 1.

Strategy (pure data parallel, 8 cores, 512 images/core):
  * Channels live on SBUF partitions. Two 64-channel image streams are
    stacked into the 128 partitions ("half0" -> partitions 0-63,
    "half1" -> 64-127) so elementwise engines run at full width.
  * x arrives from the host already zero-padded to 9x9 AND cast to bf16,
    so the input DMA lands directly in the conv1 pad tiles: no on-chip
    cast, no border memsets, no staging copy. The identity residual is
    added from the same bf16 pad tile (costs ~1e-3 rel err, budget 2e-2).
  * A 3x3 conv is 9 shifted 64x64 matmuls accumulated in PSUM; each tap
    reads a strided window of the padded tile. The 128x128 PE array is
    split into 4 64x64 quadrants via the matmul base partitions (rhs
    base -> row group, psum base -> column group); four independent
    tap-chains (2 pairs x 2 halves) keep the array fully fed.
  * BN scales are folded into the conv weights on the host; BN shifts are
    per-partition bias operands of the ACT/DVE epilogues.
  * Output returns as bf16 (host converts to f32): halves the out DMA.
  * Quad schedule ramps: a small (n=6) first quad so the first matmul
    starts as soon as one small DMA lands, and small last quads so the
    final conv2+epilogue+DMA drain is short. n=6 is the smallest quad
    whose tap-round is not LDWEIGHTS-bound.
  * The critic masks only touch batch element 0: every core runs the same
    mask multiply on its first image, but cores 1-7 get all-ones masks.
"""

import ml_dtypes
import numpy as np

import concourse.bass as bass  # noqa: F401  (engine namespaces live on the nc object)
import concourse.tile as tile
from concourse import bacc, mybir
from concourse.bass_utils import run_bass_kernel_spmd

F32 = mybir.dt.float32
BF16 = mybir.dt.bfloat16
NP_BF16 = ml_dtypes.bfloat16
EPS = 1e-5
B, C, H, W = 4096, 64, 7, 7
NCORES = 8
BPC = B // NCORES          # 512 images per core
# Pad-tile ring depth. Must be 4: at round v the quad v+2 input DMA is
# emitted before conv2(v-1)'s residual read, so their slots (v+2)%NBUF
# and (v-1)%NBUF must differ or the DMA is ordered before the read.
NBUF = 4

# Quad sizes: per quad, 4 chains of n images (2 pairs x 2 halves = 4n
# images, 2n slots). Ramped: small first quad (fast start) and small
# last quads (fast drain); sum of n = 128 (512 images / 4).
QSIZES = [6] + [10] * 11 + [6, 6]
assert sum(QSIZES) == 128
QUADS = []
_sb = 0
for _n in QSIZES:
    QUADS.append((_sb, _n))
    _sb += 2 * _n
SLOTS = _sb                # 256 slots (2 per quad-chain-image)
NMAX = max(QSIZES)

# (pair_in_quad, half, colgroup): the 4 concurrent chains of a quad.
# Even pair writes PSUM naturally, odd pair swapped - this alternation is
# what keeps all four PE quadrants busy across consecutive chains.
CHAINS = [(0, 0, 0), (1, 1, 0), (0, 1, 1), (1, 0, 1)]

_CACHE = {}


def _psum_view(psj, n):
    """[128, n, 7, 7] view of one pair's [128, 512] psum bank."""
    return psj[:, 0:n * H * W].rearrange(
        "p (i h w) -> p i h w", i=n, h=H, w=W)


def _emit_conv(nc, src_pad, w_sb, pss, n):
    """One quad of one conv: 4 concurrent 9-tap chains (36 matmuls).
    pss = per-pair [128, 512] psum banks."""
    for t in range(9):
        dh, dw = t // 3, t % 3
        for (j, half, cg) in CHAINS:
            rhs = src_pad[64 * half:64 * half + 64,
                          n * j:n * (j + 1), dh:dh + 7, dw:dw + 7]
            lhsT = w_sb[64 * half:64 * half + 64, t, :]
            out = pss[j][64 * cg:64 * cg + 64, 0:n * H * W]
            nc.tensor.matmul(out, lhsT, rhs, start=(t == 0), stop=(t == 8))


def _build():
    nc = bacc.Bacc("TRN2", target_bir_lowering=False, debug=False,
                   num_devices=NCORES)
    x_d = nc.dram_tensor("x", [128, SLOTS, 9, 9], BF16, kind="ExternalInput")
    w1_d = nc.dram_tensor("w1", [128, 9, 64], BF16, kind="ExternalInput")
    w2_d = nc.dram_tensor("w2", [128, 9, 64], BF16, kind="ExternalInput")
    # cst+msk merged into one small f32 tensor so a single early DMA on
    # the (otherwise idle) scalar HWDGE ring lands both before the first
    # epilogue: cols 0:2 = [shift1, shift2]; cols 2:100 = the two critic
    # masks (partitions 0-63 only)
    cm_d = nc.dram_tensor("cm", [128, 100], F32, kind="ExternalInput")
    o_d = nc.dram_tensor("o", [128, SLOTS, H, W], BF16, kind="ExternalOutput")

    with tile.TileContext(nc) as tc:
        with (
            tc.tile_pool(name="singles", bufs=1) as singles,
            tc.tile_pool(name="outp", bufs=3) as out_pool,
            tc.tile_pool(name="pads", bufs=1) as pad_pool,
            tc.tile_pool(name="ps1", bufs=2, space="PSUM") as ps1_pool,
            tc.tile_pool(name="ps2", bufs=2, space="PSUM") as ps2_pool,
        ):
            w1_sb = singles.tile([128, 9, 64], BF16, name="w1_sb")
            w2_sb = singles.tile([128, 9, 64], BF16, name="w2_sb")
            cm_sb = singles.tile([128, 100], F32, name="cm_sb")
            warm_sb = singles.tile([128, 1], F32, name="warm_sb")

            def shift(i):           # [128, 1] bias column for bn shift i
                return cm_sb[:, i:i + 1]

            def mask(k):            # [64, 7, 7] critic mask k
                return cm_sb[0:64, 2 + 49 * k:51 + 49 * k].rearrange(
                    "p (h w) -> p h w", h=H, w=W)

            # Persistent 9x9 pad tiles. xpads: borders arrive as zeros in
            # the host-padded DMA payload, so they are never memset.
            # y1pads: border zeroed once at startup; compute rewrites only
            # the interior.
            xpads, y1pads = [], []
            for i in range(NBUF):
                xpads.append(pad_pool.tile([128, 2 * NMAX, 9, 9], BF16,
                                           name=f"xpad{i}", tag=f"xpad{i}"))
                y1pads.append(pad_pool.tile([128, 2 * NMAX, 9, 9], BF16,
                                            name=f"y1pad{i}", tag=f"y1pad{i}"))

            def emit_in_dma(v):
                base, n = QUADS[v]
                nc.sync.dma_start(xpads[v % NBUF][:, 0:2 * n],
                                  x_d[:, base:base + 2 * n])

            def emit_in_dma0():
                # cst+msk alone on the (single-queue, ~26GB/s) scalar
                # HWDGE ring: lands ~11.3us, just ahead of the first
                # epilogue that needs it - the SWDGE route would complete
                # ~6us late behind the input flood. Quad 0's x leads the
                # 8-queue sync ring.
                nc.scalar.dma_start(cm_sb[:], cm_d[:])
                emit_in_dma(0)

            def emit_conv2(state):
                v, base, n, yp, xp = state
                # separate per-pair psum tiles: Tile deps are per-tile, so
                # a shared tile would serialize pair 0's epilogue writes
                # behind pair 1's reads (and vice versa)
                ps2 = [ps2_pool.tile([128, 512], F32, name=f"ps2{j}")
                       for j in range(2)]
                _emit_conv(nc, yp, w2_sb, ps2, n)
                if v == 0:
                    # critic mask 2 on conv2 output of batch element 0
                    tgt = ps2[0][0:64, 0:H * W].rearrange(
                        "p (h w) -> p h w", h=H, w=W)
                    nc.vector.tensor_mul(tgt, tgt, mask(1))
                # residual adds from the bf16 pad interior, back-to-back on
                # DVE; the two relu(psum+shift2) finals split DVE/ACT so
                # they run concurrently
                views = [_psum_view(ps2[j], n) for j in range(2)]
                out_q = out_pool.tile([128, 2 * NMAX, H, W], BF16,
                                      name="out_q")

                def add(j):
                    nc.vector.tensor_add(
                        views[j], views[j],
                        xp[:, n * j:n * (j + 1), 1:8, 1:8])

                def relu_p1():
                    nc.scalar.activation(
                        out=out_q[:, n:2 * n], in_=views[1],
                        func=mybir.ActivationFunctionType.Relu,
                        bias=shift(1), scale=1.0)

                def ts(j):
                    nc.vector.tensor_scalar(
                        out_q[:, n * j:n * (j + 1)], views[j],
                        shift(1), 0.0,
                        mybir.AluOpType.add, mybir.AluOpType.max)

                if v == len(QUADS) - 1:
                    # final quad: everything after the last matmul is pure
                    # drain. Pair 1's chain (DVE add -> ACT relu -> SWDGE
                    # out) races ahead; thanks to the per-pair psum tiles
                    # pair 0's DVE ops don't serialize against it. Each
                    # pair streams back separately via the gpsimd SWDGE
                    # queue - small transfers that skip the ~1.8us HWDGE
                    # kick latency right when everything is gated on the
                    # last byte landing.
                    add(1)
                    relu_p1()
                    add(0)
                    ts(0)
                    nc.gpsimd.dma_start(o_d[:, base + n:base + 2 * n],
                                        out_q[:, n:2 * n])
                    nc.gpsimd.dma_start(o_d[:, base:base + n], out_q[:, 0:n])
                else:
                    # steady state: whole epilogue on DVE (it has slack),
                    # keeping ACT free for the conv1 RELUs so the next
                    # quad's conv2 is never gated on a queued-up ACT
                    add(0)
                    add(1)
                    ts(0)
                    ts(1)
                    nc.sync.dma_start(o_d[:, base:base + 2 * n],
                                      out_q[:, 0:2 * n])

            pending = None
            for v, (base, n) in enumerate(QUADS):
                if v == 0:
                    # prologue, interleaved so no queue serializes it:
                    # quad0's x first on the HWDGE ring; w1 first on the
                    # SWDGE ring (it gates the first LDWEIGHTS); ACT table
                    # prewarm behind them; y1pad borders off-path.
                    emit_in_dma0()
                    nc.gpsimd.dma_start(w1_sb[:], w1_d[:])
                    nc.scalar.memzero(warm_sb[:])
                    nc.vector.memset(y1pads[0][:], 0.0)
                    emit_in_dma(1)
                    nc.gpsimd.dma_start(w2_sb[:], w2_d[:])
                    nc.vector.memset(y1pads[1][:], 0.0)
                    emit_in_dma(2)
                    nc.gpsimd.memset(y1pads[2][:], 0.0)
                    nc.gpsimd.memset(y1pads[3][:], 0.0)
                elif v + 2 < len(QUADS):
                    emit_in_dma(v + 2)
                xp = xpads[v % NBUF]
                ps1 = [ps1_pool.tile([128, 512], F32, name=f"ps1{j}")
                       for j in range(2)]
                _emit_conv(nc, xp, w1_sb, ps1, n)
                yp = y1pads[v % NBUF]
                for j in range(2):
                    nc.scalar.activation(
                        out=yp[:, n * j:n * (j + 1), 1:8, 1:8],
                        in_=_psum_view(ps1[j], n),
                        func=mybir.ActivationFunctionType.Relu,
                        bias=shift(0), scale=1.0)
                if v == 0:
                    # critic mask 1 on relu(bn1(conv1)) of batch elem 0
                    tgt = yp[0:64, 0, 1:8, 1:8]
                    nc.vector.tensor_mul(tgt, tgt, mask(0))
                if pending is not None:
                    emit_conv2(pending)
                pending = (v, base, n, yp, xp)
            emit_conv2(pending)

    nc.compile()
    return nc


def _get_nc():
    if "nc" not in _CACHE:
        _CACHE["nc"] = _build()
    return _CACHE["nc"]


def _host_pack(x, w1, g1, b1, m1, v1, w2, g2, b2, m2, v2, mask1, mask2):
    x = np.asarray(x, np.float32)
    scale1 = np.asarray(g1, np.float32) / np.sqrt(np.asarray(v1, np.float32) + EPS)
    shift1 = np.asarray(b1, np.float32) - np.asarray(m1, np.float32) * scale1
    scale2 = np.asarray(g2, np.float32) / np.sqrt(np.asarray(v2, np.float32) + EPS)
    shift2 = np.asarray(b2, np.float32) - np.asarray(m2, np.float32) * scale2

    def pack_w(w, scale):
        ws = np.asarray(w, np.float32) * scale[:, None, None, None]
        # [co, ci, kh, kw] -> [ci, tap, co], duplicated into both halves
        lhsT = ws.transpose(1, 2, 3, 0).reshape(64, 9, 64)
        return np.ascontiguousarray(np.tile(lhsT, (2, 1, 1)).astype(NP_BF16))

    wdev1, wdev2 = pack_w(w1, scale1), pack_w(w2, scale2)
    cst = np.tile(np.stack([shift1, shift2], 1), (2, 1)).astype(np.float32)

    def pack_cm(msk1, msk2):
        cm = np.zeros((128, 100), dtype=np.float32)
        cm[:, 0:2] = cst
        cm[0:64, 2:51] = np.asarray(msk1, np.float32).reshape(64, 49)
        cm[0:64, 51:100] = np.asarray(msk2, np.float32).reshape(64, 49)
        return np.ascontiguousarray(cm)

    # Pre-padded bf16 input: [core, 128, slot, 9, 9] with zero borders.
    # Quad q (slot base b, size n) holds images [g0, g0+4n) of its core:
    # pair-major, then half, then index -> partition half h holds channel
    # block, slot b + j*n + i.
    xb = x.reshape(NCORES, BPC, C, H, W).astype(NP_BF16)
    xdev = np.zeros((NCORES, 128, SLOTS, 9, 9), dtype=NP_BF16)
    g0 = 0
    for (sb, n) in QUADS:
        xq = xb[:, g0:g0 + 4 * n].reshape(NCORES, 2, 2, n, C, H, W)
        # [core, pair, half, i, c, h, w] -> [core, half, c, pair, i, h, w]
        xq = xq.transpose(0, 2, 4, 1, 3, 5, 6).reshape(
            NCORES, 128, 2 * n, H, W)
        xdev[:, :, sb:sb + 2 * n, 1:8, 1:8] = xq
        g0 += 4 * n

    cm0 = pack_cm(mask1, mask2)
    cm1 = pack_cm(np.ones((64, 7, 7), np.float32),
                  np.ones((64, 7, 7), np.float32))

    in_maps = []
    for c in range(NCORES):
        in_maps.append({
            "x": np.ascontiguousarray(xdev[c]),
            "w1": wdev1,
            "w2": wdev2,
            "cm": cm0 if c == 0 else cm1,
        })
    return in_maps


def _host_unpack(results):
    o = np.stack([results[c]["o"] for c in range(NCORES)]).astype(np.float32)
    out = np.empty((NCORES, BPC, C, H, W), dtype=np.float32)
    g0 = 0
    for (sb, n) in QUADS:
        oq = o[:, :, sb:sb + 2 * n].reshape(NCORES, 2, C, 2, n, H, W)
        # [core, half, c, pair, i, h, w] -> [core, pair, half, i, c, h, w]
        out[:, g0:g0 + 4 * n] = oq.transpose(0, 3, 1, 4, 2, 5, 6).reshape(
            NCORES, 4 * n, C, H, W)
        g0 += 4 * n
    return np.ascontiguousarray(out.reshape(B, C, H, W))


def run(trace=False, **inputs):
    nc = _get_nc()
    in_maps = _host_pack(**inputs)
    res = run_bass_kernel_spmd(nc, in_maps, core_ids=list(range(NCORES)),
                               trace=trace)
    return _host_unpack(res.results), res


def kernel(**inputs) -> np.ndarray:
    out, _ = run(trace=False, **inputs)
    return out


# revision 30
# speedup vs baseline: 1.0150x; 1.0150x over previous
"""Trainium2 Bass kernel: ResNet BasicBlock (conv3x3-BN-ReLU-mask-conv3x3-mask-BN-residual-ReLU).

Problem shape: x[4096, 64, 7, 7], both convs 64->64 3x3 pad 1.

Strategy (pure data parallel, 8 cores, 512 images/core):
  * Channels live on SBUF partitions. Two 64-channel image streams are
    stacked into the 128 partitions ("half0" -> partitions 0-63,
    "half1" -> 64-127) so elementwise engines run at full width.
  * x arrives from the host already zero-padded to 9x9 AND cast to bf16,
    so the input DMA lands directly in the conv1 pad tiles: no on-chip
    cast, no border memsets, no staging copy. The identity residual is
    added from the same bf16 pad tile (costs ~1e-3 rel err, budget 2e-2).
  * A 3x3 conv is 9 shifted 64x64 matmuls accumulated in PSUM; each tap
    reads a strided window of the padded tile. The 128x128 PE array is
    split into 4 64x64 quadrants via the matmul base partitions (rhs
    base -> row group, psum base -> column group); four independent
    tap-chains (2 pairs x 2 halves) keep the array fully fed.
  * BN scales are folded into the conv weights on the host; BN shifts are
    per-partition bias operands of the ACT/DVE epilogues.
  * Output returns as bf16 (host converts to f32): halves the out DMA.
  * Quad schedule ramps: a small (n=6) first quad so the first matmul
    starts as soon as one small DMA lands, and small last quads so the
    final conv2+epilogue+DMA drain is short. n=6 is the smallest quad
    whose tap-round is not LDWEIGHTS-bound.
  * The critic masks only touch batch element 0: every core runs the same
    mask multiply on its first image, but cores 1-7 get all-ones masks.
"""

import ml_dtypes
import numpy as np

import concourse.bass as bass  # noqa: F401  (engine namespaces live on the nc object)
import concourse.tile as tile
from concourse import bacc, mybir
from concourse.bass_utils import run_bass_kernel_spmd

F32 = mybir.dt.float32
BF16 = mybir.dt.bfloat16
NP_BF16 = ml_dtypes.bfloat16
EPS = 1e-5
B, C, H, W = 4096, 64, 7, 7
NCORES = 8
BPC = B // NCORES          # 512 images per core
# Pad-tile ring depth. Must be 4: at round v the quad v+2 input DMA is
# emitted before conv2(v-1)'s residual read, so their slots (v+2)%NBUF
# and (v-1)%NBUF must differ or the DMA is ordered before the read.
NBUF = 4

# Quad sizes: per quad, 4 chains of n images (2 pairs x 2 halves = 4n
# images, 2n slots). Ramped: small first quad (fast start) and small
# last quads (fast drain); sum of n = 128 (512 images / 4).
QSIZES = [6] + [10] * 11 + [6, 6]
assert sum(QSIZES) == 128
QUADS = []
_sb = 0
for _n in QSIZES:
    QUADS.append((_sb, _n))
    _sb += 2 * _n
SLOTS = _sb                # 256 slots (2 per quad-chain-image)
NMAX = max(QSIZES)

# (pair_in_quad, half, colgroup): the 4 concurrent chains of a quad.
# Even pair writes PSUM naturally, odd pair swapped - this alternation is
# what keeps all four PE quadrants busy across consecutive chains.
CHAINS = [(0, 0, 0), (1, 1, 0), (0, 1, 1), (1, 0, 1)]

_CACHE = {}


def _psum_view(psj, n):
    """[128, n, 7, 7] view of one pair's [128, 512] psum bank."""
    return psj[:, 0:n * H * W].rearrange(
        "p (i h w) -> p i h w", i=n, h=H, w=W)


def _emit_conv(nc, src_pad, w_sb, pss, n):
    """One quad of one conv: 4 concurrent 9-tap chains (36 matmuls).
    pss = per-pair [128, 512] psum banks."""
    for t in range(9):
        dh, dw = t // 3, t % 3
        for (j, half, cg) in CHAINS:
            rhs = src_pad[64 * half:64 * half + 64,
                          n * j:n * (j + 1), dh:dh + 7, dw:dw + 7]
            lhsT = w_sb[64 * half:64 * half + 64, t, :]
            out = pss[j][64 * cg:64 * cg + 64, 0:n * H * W]
            nc.tensor.matmul(out, lhsT, rhs, start=(t == 0), stop=(t == 8))


def _build():
    nc = bacc.Bacc("TRN2", target_bir_lowering=False, debug=False,
                   num_devices=NCORES)
    x_d = nc.dram_tensor("x", [128, SLOTS, 9, 9], BF16, kind="ExternalInput")
    w1_d = nc.dram_tensor("w1", [128, 9, 64], BF16, kind="ExternalInput")
    w2_d = nc.dram_tensor("w2", [128, 9, 64], BF16, kind="ExternalInput")
    # cst+msk merged into one small f32 tensor so a single early DMA on
    # the (otherwise idle) scalar HWDGE ring lands both before the first
    # epilogue: cols 0:2 = [shift1, shift2]; cols 2:100 = the two critic
    # masks (partitions 0-63 only)
    cm_d = nc.dram_tensor("cm", [128, 100], F32, kind="ExternalInput")
    o_d = nc.dram_tensor("o", [128, SLOTS, H, W], BF16, kind="ExternalOutput")

    with tile.TileContext(nc) as tc:
        with (
            tc.tile_pool(name="singles", bufs=1) as singles,
            tc.tile_pool(name="outp", bufs=3) as out_pool,
            tc.tile_pool(name="pads", bufs=1) as pad_pool,
            tc.tile_pool(name="ps1", bufs=2, space="PSUM") as ps1_pool,
            tc.tile_pool(name="ps2", bufs=2, space="PSUM") as ps2_pool,
        ):
            w1_sb = singles.tile([128, 9, 64], BF16, name="w1_sb")
            w2_sb = singles.tile([128, 9, 64], BF16, name="w2_sb")
            cm_sb = singles.tile([128, 100], F32, name="cm_sb")
            warm_sb = singles.tile([128, 1], F32, name="warm_sb")

            def shift(i):           # [128, 1] bias column for bn shift i
                return cm_sb[:, i:i + 1]

            def mask(k):            # [64, 7, 7] critic mask k
                return cm_sb[0:64, 2 + 49 * k:51 + 49 * k].rearrange(
                    "p (h w) -> p h w", h=H, w=W)

            # Persistent 9x9 pad tiles. xpads: borders arrive as zeros in
            # the host-padded DMA payload, so they are never memset.
            # y1pads: border zeroed once at startup; compute rewrites only
            # the interior.
            xpads, y1pads = [], []
            for i in range(NBUF):
                xpads.append(pad_pool.tile([128, 2 * NMAX, 9, 9], BF16,
                                           name=f"xpad{i}", tag=f"xpad{i}"))
                y1pads.append(pad_pool.tile([128, 2 * NMAX, 9, 9], BF16,
                                            name=f"y1pad{i}", tag=f"y1pad{i}"))

            def emit_in_dma(v):
                base, n = QUADS[v]
                nc.sync.dma_start(xpads[v % NBUF][:, 0:2 * n],
                                  x_d[:, base:base + 2 * n])

            def emit_in_dma0():
                # cst+msk alone on the (single-queue, ~26GB/s) scalar
                # HWDGE ring: lands ~11.3us, just ahead of the first
                # epilogue that needs it - the SWDGE route would complete
                # ~6us late behind the input flood. Quad 0's x leads the
                # 8-queue sync ring.
                nc.scalar.dma_start(cm_sb[:], cm_d[:])
                emit_in_dma(0)

            def emit_conv2(state):
                v, base, n, yp, xp = state
                # separate per-pair psum tiles: Tile deps are per-tile, so
                # a shared tile would serialize pair 0's epilogue writes
                # behind pair 1's reads (and vice versa)
                ps2 = [ps2_pool.tile([128, 512], F32, name=f"ps2{j}")
                       for j in range(2)]
                _emit_conv(nc, yp, w2_sb, ps2, n)
                if v == 0:
                    # critic mask 2 on conv2 output of batch element 0
                    tgt = ps2[0][0:64, 0:H * W].rearrange(
                        "p (h w) -> p h w", h=H, w=W)
                    nc.vector.tensor_mul(tgt, tgt, mask(1))
                # residual adds from the bf16 pad interior, back-to-back on
                # DVE; the two relu(psum+shift2) finals split DVE/ACT so
                # they run concurrently
                views = [_psum_view(ps2[j], n) for j in range(2)]
                out_q = out_pool.tile([128, 2 * NMAX, H, W], BF16,
                                      name="out_q")

                def add(j):
                    nc.vector.tensor_add(
                        views[j], views[j],
                        xp[:, n * j:n * (j + 1), 1:8, 1:8])

                def relu_p1():
                    nc.scalar.activation(
                        out=out_q[:, n:2 * n], in_=views[1],
                        func=mybir.ActivationFunctionType.Relu,
                        bias=shift(1), scale=1.0)

                def ts(j):
                    nc.vector.tensor_scalar(
                        out_q[:, n * j:n * (j + 1)], views[j],
                        shift(1), 0.0,
                        mybir.AluOpType.add, mybir.AluOpType.max)

                if v == len(QUADS) - 1:
                    # final quad: everything after the last matmul is pure
                    # drain. Pair 1's chain (DVE add -> ACT relu -> SWDGE
                    # out) races ahead; thanks to the per-pair psum tiles
                    # pair 0's DVE ops don't serialize against it. Each
                    # pair streams back separately via the gpsimd SWDGE
                    # queue - small transfers that skip the ~1.8us HWDGE
                    # kick latency right when everything is gated on the
                    # last byte landing.
                    add(1)
                    relu_p1()
                    add(0)
                    ts(0)
                    # p1 via gpsimd SWDGE, p0 via sync HWDGE: the two
                    # final transfers run on independent paths
                    nc.gpsimd.dma_start(o_d[:, base + n:base + 2 * n],
                                        out_q[:, n:2 * n])
                    nc.sync.dma_start(o_d[:, base:base + n], out_q[:, 0:n])
                else:
                    # steady state: whole epilogue on DVE (it has slack),
                    # keeping ACT free for the conv1 RELUs so the next
                    # quad's conv2 is never gated on a queued-up ACT
                    add(0)
                    add(1)
                    ts(0)
                    ts(1)
                    nc.sync.dma_start(o_d[:, base:base + 2 * n],
                                      out_q[:, 0:2 * n])

            pending = None
            for v, (base, n) in enumerate(QUADS):
                if v == 0:
                    # prologue, interleaved so no queue serializes it:
                    # quad0's x first on the HWDGE ring; w1 first on the
                    # SWDGE ring (it gates the first LDWEIGHTS); ACT table
                    # prewarm behind them; y1pad borders off-path.
                    emit_in_dma0()
                    nc.gpsimd.dma_start(w1_sb[:], w1_d[:])
                    nc.scalar.memzero(warm_sb[:])
                    nc.vector.memset(y1pads[0][:], 0.0)
                    emit_in_dma(1)
                    nc.gpsimd.dma_start(w2_sb[:], w2_d[:])
                    nc.vector.memset(y1pads[1][:], 0.0)
                    emit_in_dma(2)
                    nc.gpsimd.memset(y1pads[2][:], 0.0)
                    nc.gpsimd.memset(y1pads[3][:], 0.0)
                elif v + 2 < len(QUADS):
                    emit_in_dma(v + 2)
                xp = xpads[v % NBUF]
                ps1 = [ps1_pool.tile([128, 512], F32, name=f"ps1{j}")
                       for j in range(2)]
                _emit_conv(nc, xp, w1_sb, ps1, n)
                yp = y1pads[v % NBUF]
                for j in range(2):
                    nc.scalar.activation(
                        out=yp[:, n * j:n * (j + 1), 1:8, 1:8],
                        in_=_psum_view(ps1[j], n),
                        func=mybir.ActivationFunctionType.Relu,
                        bias=shift(0), scale=1.0)
                if v == 0:
                    # critic mask 1 on relu(bn1(conv1)) of batch elem 0
                    tgt = yp[0:64, 0, 1:8, 1:8]
                    nc.vector.tensor_mul(tgt, tgt, mask(0))
                if pending is not None:
                    emit_conv2(pending)
                pending = (v, base, n, yp, xp)
            emit_conv2(pending)

    nc.compile()
    return nc


def _get_nc():
    if "nc" not in _CACHE:
        _CACHE["nc"] = _build()
    return _CACHE["nc"]


def _host_pack(x, w1, g1, b1, m1, v1, w2, g2, b2, m2, v2, mask1, mask2):
    x = np.asarray(x, np.float32)
    scale1 = np.asarray(g1, np.float32) / np.sqrt(np.asarray(v1, np.float32) + EPS)
    shift1 = np.asarray(b1, np.float32) - np.asarray(m1, np.float32) * scale1
    scale2 = np.asarray(g2, np.float32) / np.sqrt(np.asarray(v2, np.float32) + EPS)
    shift2 = np.asarray(b2, np.float32) - np.asarray(m2, np.float32) * scale2

    def pack_w(w, scale):
        ws = np.asarray(w, np.float32) * scale[:, None, None, None]
        # [co, ci, kh, kw] -> [ci, tap, co], duplicated into both halves
        lhsT = ws.transpose(1, 2, 3, 0).reshape(64, 9, 64)
        return np.ascontiguousarray(np.tile(lhsT, (2, 1, 1)).astype(NP_BF16))

    wdev1, wdev2 = pack_w(w1, scale1), pack_w(w2, scale2)
    cst = np.tile(np.stack([shift1, shift2], 1), (2, 1)).astype(np.float32)

    def pack_cm(msk1, msk2):
        cm = np.zeros((128, 100), dtype=np.float32)
        cm[:, 0:2] = cst
        cm[0:64, 2:51] = np.asarray(msk1, np.float32).reshape(64, 49)
        cm[0:64, 51:100] = np.asarray(msk2, np.float32).reshape(64, 49)
        return np.ascontiguousarray(cm)

    # Pre-padded bf16 input: [core, 128, slot, 9, 9] with zero borders.
    # Quad q (slot base b, size n) holds images [g0, g0+4n) of its core:
    # pair-major, then half, then index -> partition half h holds channel
    # block, slot b + j*n + i.
    xb = x.reshape(NCORES, BPC, C, H, W).astype(NP_BF16)
    xdev = np.zeros((NCORES, 128, SLOTS, 9, 9), dtype=NP_BF16)
    g0 = 0
    for (sb, n) in QUADS:
        xq = xb[:, g0:g0 + 4 * n].reshape(NCORES, 2, 2, n, C, H, W)
        # [core, pair, half, i, c, h, w] -> [core, half, c, pair, i, h, w]
        xq = xq.transpose(0, 2, 4, 1, 3, 5, 6).reshape(
            NCORES, 128, 2 * n, H, W)
        xdev[:, :, sb:sb + 2 * n, 1:8, 1:8] = xq
        g0 += 4 * n

    cm0 = pack_cm(mask1, mask2)
    cm1 = pack_cm(np.ones((64, 7, 7), np.float32),
                  np.ones((64, 7, 7), np.float32))

    in_maps = []
    for c in range(NCORES):
        in_maps.append({
            "x": np.ascontiguousarray(xdev[c]),
            "w1": wdev1,
            "w2": wdev2,
            "cm": cm0 if c == 0 else cm1,
        })
    return in_maps


def _host_unpack(results):
    o = np.stack([results[c]["o"] for c in range(NCORES)]).astype(np.float32)
    out = np.empty((NCORES, BPC, C, H, W), dtype=np.float32)
    g0 = 0
    for (sb, n) in QUADS:
        oq = o[:, :, sb:sb + 2 * n].reshape(NCORES, 2, C, 2, n, H, W)
        # [core, half, c, pair, i, h, w] -> [core, pair, half, i, c, h, w]
        out[:, g0:g0 + 4 * n] = oq.transpose(0, 3, 1, 4, 2, 5, 6).reshape(
            NCORES, 4 * n, C, H, W)
        g0 += 4 * n
    return np.ascontiguousarray(out.reshape(B, C, H, W))


def run(trace=False, **inputs):
    nc = _get_nc()
    in_maps = _host_pack(**inputs)
    res = run_bass_kernel_spmd(nc, in_maps, core_ids=list(range(NCORES)),
                               trace=trace)
    return _host_unpack(res.results), res


def kernel(**inputs) -> np.ndarray:
    out, _ = run(trace=False, **inputs)
    return out


# revision 31
# speedup vs baseline: 1.0230x; 1.0078x over previous
"""Trainium2 Bass kernel: ResNet BasicBlock (conv3x3-BN-ReLU-mask-conv3x3-mask-BN-residual-ReLU).

Problem shape: x[4096, 64, 7, 7], both convs 64->64 3x3 pad 1.

Strategy (pure data parallel, 8 cores, 512 images/core):
  * Channels live on SBUF partitions. Two 64-channel image streams are
    stacked into the 128 partitions ("half0" -> partitions 0-63,
    "half1" -> 64-127) so elementwise engines run at full width.
  * x arrives from the host already zero-padded to 9x9 AND cast to bf16,
    so the input DMA lands directly in the conv1 pad tiles: no on-chip
    cast, no border memsets, no staging copy. The identity residual is
    added from the same bf16 pad tile (costs ~1e-3 rel err, budget 2e-2).
  * A 3x3 conv is 9 shifted 64x64 matmuls accumulated in PSUM; each tap
    reads a strided window of the padded tile. The 128x128 PE array is
    split into 4 64x64 quadrants via the matmul base partitions (rhs
    base -> row group, psum base -> column group); four independent
    tap-chains (2 pairs x 2 halves) keep the array fully fed.
  * BN scales are folded into the conv weights on the host; BN shifts are
    per-partition bias operands of the ACT/DVE epilogues.
  * Output returns as bf16 (host converts to f32): halves the out DMA.
  * Quad schedule ramps: a small (n=6) first quad so the first matmul
    starts as soon as one small DMA lands, and small last quads so the
    final conv2+epilogue+DMA drain is short. n=6 is the smallest quad
    whose tap-round is not LDWEIGHTS-bound.
  * The critic masks only touch batch element 0: every core runs the same
    mask multiply on its first image, but cores 1-7 get all-ones masks.
"""

import ml_dtypes
import numpy as np

import concourse.bass as bass  # noqa: F401  (engine namespaces live on the nc object)
import concourse.tile as tile
from concourse import bacc, mybir
from concourse.bass_utils import run_bass_kernel_spmd

F32 = mybir.dt.float32
BF16 = mybir.dt.bfloat16
NP_BF16 = ml_dtypes.bfloat16
EPS = 1e-5
B, C, H, W = 4096, 64, 7, 7
NCORES = 8
BPC = B // NCORES          # 512 images per core
# Pad-tile ring depth. Must be 4: at round v the quad v+2 input DMA is
# emitted before conv2(v-1)'s residual read, so their slots (v+2)%NBUF
# and (v-1)%NBUF must differ or the DMA is ordered before the read.
NBUF = 4

# Quad sizes: per quad, 4 chains of n images (2 pairs x 2 halves = 4n
# images, 2n slots). Ramped: small first quad (fast start) and small
# last quads (fast drain); sum of n = 128 (512 images / 4).
QSIZES = [6] + [10] * 11 + [6, 6]
assert sum(QSIZES) == 128
QUADS = []
_sb = 0
for _n in QSIZES:
    QUADS.append((_sb, _n))
    _sb += 2 * _n
SLOTS = _sb                # 256 slots (2 per quad-chain-image)
NMAX = max(QSIZES)

# (pair_in_quad, half, colgroup): the 4 concurrent chains of a quad.
# Even pair writes PSUM naturally, odd pair swapped - this alternation is
# what keeps all four PE quadrants busy across consecutive chains.
CHAINS = [(0, 0, 0), (1, 1, 0), (0, 1, 1), (1, 0, 1)]

_CACHE = {}


def _psum_view(psj, n):
    """[128, n, 7, 7] view of one pair's [128, 512] psum bank."""
    return psj[:, 0:n * H * W].rearrange(
        "p (i h w) -> p i h w", i=n, h=H, w=W)


def _emit_conv(nc, src_pad, w_sb, pss, n):
    """One quad of one conv: 4 concurrent 9-tap chains (36 matmuls).
    pss = per-pair [128, 512] psum banks."""
    for t in range(9):
        dh, dw = t // 3, t % 3
        for (j, half, cg) in CHAINS:
            rhs = src_pad[64 * half:64 * half + 64,
                          n * j:n * (j + 1), dh:dh + 7, dw:dw + 7]
            lhsT = w_sb[64 * half:64 * half + 64, t, :]
            out = pss[j][64 * cg:64 * cg + 64, 0:n * H * W]
            nc.tensor.matmul(out, lhsT, rhs, start=(t == 0), stop=(t == 8))


def _build():
    nc = bacc.Bacc("TRN2", target_bir_lowering=False, debug=False,
                   num_devices=NCORES)
    x_d = nc.dram_tensor("x", [128, SLOTS, 9, 9], BF16, kind="ExternalInput")
    w1_d = nc.dram_tensor("w1", [128, 9, 64], BF16, kind="ExternalInput")
    w2_d = nc.dram_tensor("w2", [128, 9, 64], BF16, kind="ExternalInput")
    # cst+msk merged into one small f32 tensor so a single early DMA on
    # the (otherwise idle) scalar HWDGE ring lands both before the first
    # epilogue: cols 0:2 = [shift1, shift2]; cols 2:100 = the two critic
    # masks (partitions 0-63 only)
    cm_d = nc.dram_tensor("cm", [128, 100], F32, kind="ExternalInput")
    o_d = nc.dram_tensor("o", [128, SLOTS, H, W], BF16, kind="ExternalOutput")

    with tile.TileContext(nc) as tc:
        with (
            tc.tile_pool(name="singles", bufs=1) as singles,
            tc.tile_pool(name="outp", bufs=3) as out_pool,
            tc.tile_pool(name="pads", bufs=1) as pad_pool,
            tc.tile_pool(name="ps1", bufs=2, space="PSUM") as ps1_pool,
            tc.tile_pool(name="ps2", bufs=2, space="PSUM") as ps2_pool,
        ):
            w1_sb = singles.tile([128, 9, 64], BF16, name="w1_sb")
            w2_sb = singles.tile([128, 9, 64], BF16, name="w2_sb")
            cm_sb = singles.tile([128, 100], F32, name="cm_sb")
            warm_sb = singles.tile([128, 1], F32, name="warm_sb")

            def shift(i):           # [128, 1] bias column for bn shift i
                return cm_sb[:, i:i + 1]

            def mask(k):            # [64, 7, 7] critic mask k
                return cm_sb[0:64, 2 + 49 * k:51 + 49 * k].rearrange(
                    "p (h w) -> p h w", h=H, w=W)

            # Persistent 9x9 pad tiles. xpads: borders arrive as zeros in
            # the host-padded DMA payload, so they are never memset.
            # y1pads: border zeroed once at startup; compute rewrites only
            # the interior.
            xpads, y1pads = [], []
            for i in range(NBUF):
                xpads.append(pad_pool.tile([128, 2 * NMAX, 9, 9], BF16,
                                           name=f"xpad{i}", tag=f"xpad{i}"))
                y1pads.append(pad_pool.tile([128, 2 * NMAX, 9, 9], BF16,
                                            name=f"y1pad{i}", tag=f"y1pad{i}"))

            def emit_in_dma(v):
                base, n = QUADS[v]
                nc.sync.dma_start(xpads[v % NBUF][:, 0:2 * n],
                                  x_d[:, base:base + 2 * n])

            def emit_in_dma0():
                # cst+msk alone on the (single-queue, ~26GB/s) scalar
                # HWDGE ring: lands ~11.3us, just ahead of the first
                # epilogue that needs it - the SWDGE route would complete
                # ~6us late behind the input flood. Quad 0's x leads the
                # sync ring as two dma_starts (two HWDGE queues in
                # parallel) since it gates the first matmul.
                nc.scalar.dma_start(cm_sb[:], cm_d[:])
                base, n = QUADS[0]
                nc.sync.dma_start(xpads[0][0:64, 0:2 * n],
                                  x_d[0:64, base:base + 2 * n])
                nc.sync.dma_start(xpads[0][64:128, 0:2 * n],
                                  x_d[64:128, base:base + 2 * n])

            def emit_conv2(state):
                v, base, n, yp, xp = state
                # separate per-pair psum tiles: Tile deps are per-tile, so
                # a shared tile would serialize pair 0's epilogue writes
                # behind pair 1's reads (and vice versa)
                ps2 = [ps2_pool.tile([128, 512], F32, name=f"ps2{j}")
                       for j in range(2)]
                _emit_conv(nc, yp, w2_sb, ps2, n)
                if v == 0:
                    # critic mask 2 on conv2 output of batch element 0
                    tgt = ps2[0][0:64, 0:H * W].rearrange(
                        "p (h w) -> p h w", h=H, w=W)
                    nc.vector.tensor_mul(tgt, tgt, mask(1))
                # residual adds from the bf16 pad interior, back-to-back on
                # DVE; the two relu(psum+shift2) finals split DVE/ACT so
                # they run concurrently
                views = [_psum_view(ps2[j], n) for j in range(2)]
                out_q = out_pool.tile([128, 2 * NMAX, H, W], BF16,
                                      name="out_q")

                def add(j):
                    nc.vector.tensor_add(
                        views[j], views[j],
                        xp[:, n * j:n * (j + 1), 1:8, 1:8])

                def relu_p1():
                    nc.scalar.activation(
                        out=out_q[:, n:2 * n], in_=views[1],
                        func=mybir.ActivationFunctionType.Relu,
                        bias=shift(1), scale=1.0)

                def ts(j):
                    nc.vector.tensor_scalar(
                        out_q[:, n * j:n * (j + 1)], views[j],
                        shift(1), 0.0,
                        mybir.AluOpType.add, mybir.AluOpType.max)

                if v == len(QUADS) - 1:
                    # final quad: everything after the last matmul is pure
                    # drain. Pair 1's chain (DVE add -> ACT relu -> SWDGE
                    # out) races ahead; thanks to the per-pair psum tiles
                    # pair 0's DVE ops don't serialize against it. Each
                    # pair streams back separately via the gpsimd SWDGE
                    # queue - small transfers that skip the ~1.8us HWDGE
                    # kick latency right when everything is gated on the
                    # last byte landing.
                    add(1)
                    relu_p1()
                    add(0)
                    ts(0)
                    # p1 via gpsimd SWDGE, p0 via sync HWDGE: the two
                    # final transfers run on independent paths
                    nc.gpsimd.dma_start(o_d[:, base + n:base + 2 * n],
                                        out_q[:, n:2 * n])
                    nc.sync.dma_start(o_d[:, base:base + n], out_q[:, 0:n])
                else:
                    # steady state: whole epilogue on DVE (it has slack),
                    # keeping ACT free for the conv1 RELUs so the next
                    # quad's conv2 is never gated on a queued-up ACT
                    add(0)
                    add(1)
                    ts(0)
                    ts(1)
                    nc.sync.dma_start(o_d[:, base:base + 2 * n],
                                      out_q[:, 0:2 * n])

            pending = None
            for v, (base, n) in enumerate(QUADS):
                if v == 0:
                    # prologue, interleaved so no queue serializes it:
                    # quad0's x first on the HWDGE ring; w1 first on the
                    # SWDGE ring (it gates the first LDWEIGHTS); ACT table
                    # prewarm behind them; y1pad borders off-path.
                    emit_in_dma0()
                    nc.gpsimd.dma_start(w1_sb[:], w1_d[:])
                    nc.scalar.memzero(warm_sb[:])
                    nc.vector.memset(y1pads[0][:], 0.0)
                    emit_in_dma(1)
                    nc.gpsimd.dma_start(w2_sb[:], w2_d[:])
                    nc.vector.memset(y1pads[1][:], 0.0)
                    emit_in_dma(2)
                    nc.gpsimd.memset(y1pads[2][:], 0.0)
                    nc.gpsimd.memset(y1pads[3][:], 0.0)
                elif v + 2 < len(QUADS):
                    emit_in_dma(v + 2)
                xp = xpads[v % NBUF]
                ps1 = [ps1_pool.tile([128, 512], F32, name=f"ps1{j}")
                       for j in range(2)]
                _emit_conv(nc, xp, w1_sb, ps1, n)
                yp = y1pads[v % NBUF]
                for j in range(2):
                    nc.scalar.activation(
                        out=yp[:, n * j:n * (j + 1), 1:8, 1:8],
                        in_=_psum_view(ps1[j], n),
                        func=mybir.ActivationFunctionType.Relu,
                        bias=shift(0), scale=1.0)
                if v == 0:
                    # critic mask 1 on relu(bn1(conv1)) of batch elem 0
                    tgt = yp[0:64, 0, 1:8, 1:8]
                    nc.vector.tensor_mul(tgt, tgt, mask(0))
                if pending is not None:
                    emit_conv2(pending)
                pending = (v, base, n, yp, xp)
            emit_conv2(pending)

    nc.compile()
    return nc


def _get_nc():
    if "nc" not in _CACHE:
        _CACHE["nc"] = _build()
    return _CACHE["nc"]


def _host_pack(x, w1, g1, b1, m1, v1, w2, g2, b2, m2, v2, mask1, mask2):
    x = np.asarray(x, np.float32)
    scale1 = np.asarray(g1, np.float32) / np.sqrt(np.asarray(v1, np.float32) + EPS)
    shift1 = np.asarray(b1, np.float32) - np.asarray(m1, np.float32) * scale1
    scale2 = np.asarray(g2, np.float32) / np.sqrt(np.asarray(v2, np.float32) + EPS)
    shift2 = np.asarray(b2, np.float32) - np.asarray(m2, np.float32) * scale2

    def pack_w(w, scale):
        ws = np.asarray(w, np.float32) * scale[:, None, None, None]
        # [co, ci, kh, kw] -> [ci, tap, co], duplicated into both halves
        lhsT = ws.transpose(1, 2, 3, 0).reshape(64, 9, 64)
        return np.ascontiguousarray(np.tile(lhsT, (2, 1, 1)).astype(NP_BF16))

    wdev1, wdev2 = pack_w(w1, scale1), pack_w(w2, scale2)
    cst = np.tile(np.stack([shift1, shift2], 1), (2, 1)).astype(np.float32)

    def pack_cm(msk1, msk2):
        cm = np.zeros((128, 100), dtype=np.float32)
        cm[:, 0:2] = cst
        cm[0:64, 2:51] = np.asarray(msk1, np.float32).reshape(64, 49)
        cm[0:64, 51:100] = np.asarray(msk2, np.float32).reshape(64, 49)
        return np.ascontiguousarray(cm)

    # Pre-padded bf16 input: [core, 128, slot, 9, 9] with zero borders.
    # Quad q (slot base b, size n) holds images [g0, g0+4n) of its core:
    # pair-major, then half, then index -> partition half h holds channel
    # block, slot b + j*n + i.
    xb = x.reshape(NCORES, BPC, C, H, W).astype(NP_BF16)
    xdev = np.zeros((NCORES, 128, SLOTS, 9, 9), dtype=NP_BF16)
    g0 = 0
    for (sb, n) in QUADS:
        xq = xb[:, g0:g0 + 4 * n].reshape(NCORES, 2, 2, n, C, H, W)
        # [core, pair, half, i, c, h, w] -> [core, half, c, pair, i, h, w]
        xq = xq.transpose(0, 2, 4, 1, 3, 5, 6).reshape(
            NCORES, 128, 2 * n, H, W)
        xdev[:, :, sb:sb + 2 * n, 1:8, 1:8] = xq
        g0 += 4 * n

    cm0 = pack_cm(mask1, mask2)
    cm1 = pack_cm(np.ones((64, 7, 7), np.float32),
                  np.ones((64, 7, 7), np.float32))

    in_maps = []
    for c in range(NCORES):
        in_maps.append({
            "x": np.ascontiguousarray(xdev[c]),
            "w1": wdev1,
            "w2": wdev2,
            "cm": cm0 if c == 0 else cm1,
        })
    return in_maps


def _host_unpack(results):
    o = np.stack([results[c]["o"] for c in range(NCORES)]).astype(np.float32)
    out = np.empty((NCORES, BPC, C, H, W), dtype=np.float32)
    g0 = 0
    for (sb, n) in QUADS:
        oq = o[:, :, sb:sb + 2 * n].reshape(NCORES, 2, C, 2, n, H, W)
        # [core, half, c, pair, i, h, w] -> [core, pair, half, i, c, h, w]
        out[:, g0:g0 + 4 * n] = oq.transpose(0, 3, 1, 4, 2, 5, 6).reshape(
            NCORES, 4 * n, C, H, W)
        g0 += 4 * n
    return np.ascontiguousarray(out.reshape(B, C, H, W))


def run(trace=False, **inputs):
    nc = _get_nc()
    in_maps = _host_pack(**inputs)
    res = run_bass_kernel_spmd(nc, in_maps, core_ids=list(range(NCORES)),
                               trace=trace)
    return _host_unpack(res.results), res


def kernel(**inputs) -> np.ndarray:
    out, _ = run(trace=False, **inputs)
    return out
